# revision 24
# baseline (speedup 1.0000x reference)
"""KMeansProbSampler Trainium2 kernel (8-core SPMD), v3.

Algorithm (per reference): 8 iterations of
  d2[p,c]   = (h_p - a_c)^2 + (w_p - b_c)^2        (pixel grid 1024x1024, C=128)
  assign[p] = argmin_c max(1, sqrt(d2))            (first-index tie break)
  new[c]    = sum_{p: assign==c} coords_p * heatmap_p / max(1, sqrt(min d2))

Mapping (per core: 128 pixel rows, 1024 columns; tile = one column):
  - dist: split-bf16 expansion, K=8 per tile, 4 tiles packed per matmul
    (K=32, N=512, block-diagonal ext) at 1 cyc/row.  Pixel features
    [h', h', w', w', hw2_hi, hw2_lo, 1, 1] (h' per-core, w' per-block
    recentered; all exactly bf16) x cluster features
    [-2a'_hi, -2a'_lo, -2b'_hi, -2b'_lo, 1, 1, s2_hi, s2_lo]
    (hi/lo bf16 splits; d2 error ~1e-1).
  - m2n = -min_c d2 per tile: DVE tensor_reduce(min, negate) straight from
    PSUM fp32 (exact).
  - one-hot: for 14 of each 16 tiles the SCALAR engine computes the
    COMPLEMENT notho = Sign(d2 - m2) in {0,1} (exact fp32 compare via
    per-partition bias); the other 2 tiles use a DVE tensor_scalar
    ((d2 + m2n) is_equal 0) positive one-hot.  This splits the compare
    pass across two engines.
  - rec = 1/max(1, sqrt(m2)) is folded into the scatter weights:
    vs[p, (t,d)] = vhw * rec (one small TT per 16 tiles).
  - scatter: complement tiles accumulate accN[2,c] += vs_t^T @ notho_t and
    a ones-matmul accumulates T[2,(t,d)] = column sums of vs over the same
    tiles; positive tiles accumulate accP.  True sums = accP + T - accN,
    fixed up after the per-iteration AllReduce of the raw [2, 284] strip
    (all three accumulators are linear in the per-core data).
  - duplicate clusters get +1e30 in d2 -> complement 1 everywhere -> their
    fixed-up sum is exactly 0, matching reference empty-cluster behavior.
"""

import os
import sys

import numpy as np

H = 1024
W = 1024
C = 128
N_ITER = 8
NCORES = 8
RPC = H // NCORES  # rows per core
P = 128            # partitions = pixels per tile
NT = W             # tiles (columns) per core
GT = 4             # tiles per matmul group
TPB = 128          # tiles per w-block
CHUNK = 16         # groups per pixT DMA chunk
NDVE = 2           # tiles per 8-batch handled by DVE (slots 0..NDVE-1)
BIG = 1.0e30       # duplicate-cluster mask

_REPO_CANDIDATES = ("/opt/trn_rl_repo", "/root/.axon_site/_ro/trn_rl_repo")


def _ensure_repo():
    try:
        import concourse  # noqa: F401
        return
    except ImportError:
        pass
    for p in _REPO_CANDIDATES:
        if os.path.isdir(p):
            sys.path.insert(0, p)
            break
    import concourse  # noqa: F401


def build_nc(n_iter: int = N_ITER, nt: int = NT, ncores: int = NCORES):
    """Build the SPMD Bass program (same program for every core)."""
    _ensure_repo()
    import concourse.bacc as bacc
    import concourse.mybir as mybir
    import concourse.tile as tile

    f32 = mybir.dt.float32
    f16 = mybir.dt.float16
    bf16 = mybir.dt.bfloat16
    Alu = mybir.AluOpType
    Act = mybir.ActivationFunctionType
    X = mybir.AxisListType.X

    ng = nt // GT                  # matmul groups
    nbat = ng // 2                 # 8-tile batches (2 groups each)
    nblk = (nt + TPB - 1) // TPB   # w-blocks
    assert nt % TPB == 0 and ng % 2 == 0
    nact = 8 - NDVE                # ACT (complement) tiles per batch
    t_last_dve = nt - nact - 1     # last tile with slot < NDVE

    nc = bacc.Bacc(
        "TRN2",
        target_bir_lowering=False,
        debug=False,
        num_devices=ncores,
    )

    # ---- I/O ----
    pixT_d = nc.dram_tensor("pixT", [32, ng * P], bf16, kind="ExternalInput")
    vhwT_d = nc.dram_tensor("vhwT", [P, 2 * nt], f16, kind="ExternalInput")
    clus_d = nc.dram_tensor("clus", [C, 2], f32, kind="ExternalInput")
    chv_d = nc.dram_tensor("chv", [C, 1], f32, kind="ExternalInput")
    wB_d = nc.dram_tensor("wB", [C, nblk], f32, kind="ExternalInput")
    ident_d = nc.dram_tensor("ident", [P, P], f32, kind="ExternalInput")
    ltri_d = nc.dram_tensor("ltri", [P, P], f32, kind="ExternalInput")
    out_d = nc.dram_tensor("out", [C, 2], f32, kind="ExternalOutput")

    ACCW = 256 + 2 * nact          # acc strip: accN | accP | accT

    with tile.TileContext(nc) as tc:
        from contextlib import ExitStack

        with ExitStack() as st:
            const = st.enter_context(tc.tile_pool(name="const", bufs=1))
            stpool = st.enter_context(tc.tile_pool(name="stage", bufs=2))
            sohp = st.enter_context(tc.tile_pool(name="soh", bufs=6))
            m2p = st.enter_context(tc.tile_pool(name="m2", bufs=3))
            sqp = st.enter_context(tc.tile_pool(name="sq", bufs=2))
            recp = st.enter_context(tc.tile_pool(name="rec", bufs=2))
            vsp = st.enter_context(tc.tile_pool(name="vs", bufs=3))
            smal = st.enter_context(tc.tile_pool(name="small", bufs=4))
            eqp = st.enter_context(tc.tile_pool(name="eq", bufs=2))
            psd = st.enter_context(tc.tile_pool(name="psd", bufs=3, space="PSUM"))
            psa = st.enter_context(tc.tile_pool(name="psa", bufs=1, space="PSUM"))
            pse = st.enter_context(tc.tile_pool(name="pse", bufs=1, space="PSUM"))
            dram = st.enter_context(tc.tile_pool(name="dram", bufs=2, space="DRAM"))

            # ---- persistent SBUF state ----
            vhwT = const.tile([P, 2 * nt], f16)
            ident = const.tile([P, P], f32)
            ltri = const.tile([P, P], f32)
            chv = const.tile([C, 1], f32)
            wB = const.tile([C, nblk], f32)
            clus0 = const.tile([C, 2], f32)
            exF = const.tile([C, 8 * nblk], f32)
            rhsAll = const.tile([32, 512 * nblk], bf16)
            ones2 = const.tile([P, 2], f16)

            nc.gpsimd.dma_start(vhwT[:], vhwT_d[:])
            nc.gpsimd.dma_start(ident[:], ident_d[:])
            nc.gpsimd.dma_start(ltri[:], ltri_d[:])
            nc.gpsimd.dma_start(chv[:], chv_d[:])
            nc.gpsimd.dma_start(wB[:], wB_d[:])
            nc.gpsimd.dma_start(clus0[:], clus_d[:])

            nc.vector.memset(rhsAll[:], 0.0)
            nc.vector.memset(ones2[:], 1.0)
            for b in range(nblk):
                nc.vector.memset(exF[:, 8 * b + 4:8 * b + 6], 1.0)

            def build_ext(ncs):
                """Emit ops building rhsAll (block-diag bf16 ext) from ncs [C,2] f32."""
                # duplicate-cluster detection (first duplicate wins)
                abc = pse.tile([C, C], f32, space="PSUM", tag="ep")
                nc.tensor.transpose(
                    out=abc[:], in_=ncs[:, 0:1].to_broadcast([C, C]),
                    identity=ident[:],
                )
                eqa = eqp.tile([C, C], f32, tag="eqa")
                nc.vector.tensor_scalar(
                    out=eqa[:], in0=abc[:], scalar1=ncs[:, 0:1], scalar2=None,
                    op0=Alu.is_equal,
                )
                bbc = pse.tile([C, C], f32, space="PSUM", tag="ep")
                nc.tensor.transpose(
                    out=bbc[:], in_=ncs[:, 1:2].to_broadcast([C, C]),
                    identity=ident[:],
                )
                eqb = eqp.tile([C, C], f32, tag="eqb")
                nc.vector.tensor_scalar(
                    out=eqb[:], in0=bbc[:], scalar1=ncs[:, 1:2], scalar2=None,
                    op0=Alu.is_equal,
                )
                nc.vector.tensor_tensor(out=eqa[:], in0=eqa[:], in1=eqb[:],
                                        op=Alu.mult)
                nc.vector.tensor_tensor(out=eqa[:], in0=eqa[:], in1=ltri[:],
                                        op=Alu.mult)
                cfs = smal.tile([C, 1], f32, tag="cfs")
                nc.vector.tensor_reduce(out=cfs[:], in_=eqa[:], axis=X,
                                        op=Alu.add)

                # a-part: a' = a - ch; -2a' split hi/lo; s2base = a'^2 + BIG*dup
                aP = smal.tile([C, 1], f32, tag="aP")
                nc.vector.tensor_scalar(out=aP[:], in0=ncs[:, 0:1],
                                        scalar1=chv[:, 0:1], scalar2=None,
                                        op0=Alu.subtract)
                m2a = smal.tile([C, 1], f32, tag="m2a")
                nc.vector.tensor_scalar(out=m2a[:], in0=aP[:], scalar1=-2.0,
                                        scalar2=None, op0=Alu.mult)
                m2a_hb = smal.tile([C, 1], bf16, tag="m2a_hb")
                nc.vector.tensor_copy(out=m2a_hb[:], in_=m2a[:])
                m2a_hf = smal.tile([C, 1], f32, tag="m2a_hf")
                nc.vector.tensor_copy(out=m2a_hf[:], in_=m2a_hb[:])
                m2a_lo = smal.tile([C, 1], f32, tag="m2a_lo")
                nc.vector.tensor_tensor(out=m2a_lo[:], in0=m2a[:],
                                        in1=m2a_hf[:], op=Alu.subtract)
                s2b = smal.tile([C, 1], f32, tag="s2b")
                nc.vector.tensor_tensor(out=s2b[:], in0=aP[:], in1=aP[:],
                                        op=Alu.mult)
                nc.vector.tensor_scalar(out=cfs[:], in0=cfs[:], scalar1=BIG,
                                        scalar2=None, op0=Alu.mult)
                nc.vector.tensor_tensor(out=s2b[:], in0=s2b[:], in1=cfs[:],
                                        op=Alu.add)

                # b-part, all blocks at once: [C, nblk]
                bP8 = smal.tile([C, nblk], f32, tag="bP8")
                nc.vector.tensor_tensor(
                    out=bP8[:], in0=ncs[:, 1:2].to_broadcast([C, nblk]),
                    in1=wB[:], op=Alu.subtract)
                m2b8 = smal.tile([C, nblk], f32, tag="m2b8")
                nc.vector.tensor_scalar(out=m2b8[:], in0=bP8[:], scalar1=-2.0,
                                        scalar2=None, op0=Alu.mult)
                m2b8_hb = smal.tile([C, nblk], bf16, tag="m2b8_hb")
                nc.vector.tensor_copy(out=m2b8_hb[:], in_=m2b8[:])
                m2b8_hf = smal.tile([C, nblk], f32, tag="m2b8_hf")
                nc.vector.tensor_copy(out=m2b8_hf[:], in_=m2b8_hb[:])
                m2b8_lo = smal.tile([C, nblk], f32, tag="m2b8_lo")
                nc.vector.tensor_tensor(out=m2b8_lo[:], in0=m2b8[:],
                                        in1=m2b8_hf[:], op=Alu.subtract)
                s28 = smal.tile([C, nblk], f32, tag="s28")
                nc.vector.tensor_tensor(out=s28[:], in0=bP8[:], in1=bP8[:],
                                        op=Alu.mult)
                nc.vector.tensor_tensor(out=s28[:], in0=s28[:],
                                        in1=s2b[:].to_broadcast([C, nblk]),
                                        op=Alu.add)
                s28_hb = smal.tile([C, nblk], bf16, tag="s28_hb")
                nc.vector.tensor_copy(out=s28_hb[:], in_=s28[:])
                s28_hf = smal.tile([C, nblk], f32, tag="s28_hf")
                nc.vector.tensor_copy(out=s28_hf[:], in_=s28_hb[:])
                s28_lo = smal.tile([C, nblk], f32, tag="s28_lo")
                nc.vector.tensor_tensor(out=s28_lo[:], in0=s28[:],
                                        in1=s28_hf[:], op=Alu.subtract)

                # scatter into exF [C, (b, k)] strided views
                exV = exF[:].rearrange("p (b k) -> p b k", k=8)
                nc.vector.tensor_copy(out=exV[:, :, 0:1],
                                      in_=m2a_hf[:].to_broadcast([C, nblk, 1]))
                nc.vector.tensor_copy(out=exV[:, :, 1:2],
                                      in_=m2a_lo[:].to_broadcast([C, nblk, 1]))
                nc.vector.tensor_copy(out=exV[:, :, 2:3],
                                      in_=m2b8_hf[:].unsqueeze(2))
                nc.vector.tensor_copy(out=exV[:, :, 3:4],
                                      in_=m2b8_lo[:].unsqueeze(2))
                nc.vector.tensor_copy(out=exV[:, :, 6:7],
                                      in_=s28_hf[:].unsqueeze(2))
                nc.vector.tensor_copy(out=exV[:, :, 7:8],
                                      in_=s28_lo[:].unsqueeze(2))

                # transpose -> [8*nblk, C], cast bf16, scatter into rhsAll
                extp = pse.tile([8 * nblk, C], f32, space="PSUM", tag="ep")
                nc.tensor.transpose(out=extp[:], in_=exF[:], identity=ident[:])
                extb = smal.tile([8 * nblk, C], bf16, tag="extb")
                nc.scalar.copy(out=extb[:], in_=extp[:])
                for b in range(nblk):
                    for s in range(GT):
                        nc.gpsimd.dma_start(
                            rhsAll[8 * s:8 * s + 8,
                                   512 * b + P * s:512 * b + P * (s + 1)],
                            extb[8 * b:8 * b + 8, :])

            build_ext(clus0)

            for it in range(n_iter):
                acc = psa.tile([2, 512], f32, space="PSUM")
                stage = None
                pendR = []  # (pi, m2n, scat) awaiting rec-chain
                pendF = []  # (pi, vs, scat) awaiting scatter flush

                def rec_chain(entry):
                    pi, m2n_e, scat = entry
                    sq = sqp.tile([P, 16], f32, tag="sq")
                    nc.vector.tensor_scalar(
                        out=sq[:], in0=m2n_e[:], scalar1=1.0,
                        scalar2=None, op0=Alu.max,
                    )
                    nc.scalar.activation(out=sq[:], in_=sq[:], func=Act.Sqrt)
                    rec = recp.tile([P, 16], f16, tag="rec")
                    with nc.allow_low_precision(reason="f16 weights"):
                        nc.vector.reciprocal(out=rec[:], in_=sq[:])
                    vs = vsp.tile([P, 32], f16, tag="vs")
                    nc.vector.tensor_tensor(
                        out=vs[:].rearrange("p (t d) -> p t d", d=2),
                        in0=vhwT[:, 32 * pi:32 * pi + 32].rearrange(
                            "p (t d) -> p t d", d=2),
                        in1=rec[:].unsqueeze(2).to_broadcast([P, 16, 2]),
                        op=Alu.mult,
                    )
                    return (pi, vs, scat)

                def flush(entry):
                    pi, vs_f, scat = entry
                    for q2 in range(2):
                        nc.tensor.matmul(
                            out=acc[:, 256:256 + 2 * nact],
                            lhsT=ones2[:],
                            rhs=vs_f[:, 16 * q2 + 2 * NDVE:16 * q2 + 16],
                            start=(pi == 0 and q2 == 0),
                            stop=(pi == nbat // 2 - 1 and q2 == 1),
                        )
                    for soh_ap, vcol, region, last in scat:
                        nc.tensor.matmul(out=region, lhsT=vs_f[:, vcol:vcol + 2],
                                         rhs=soh_ap, start=False, stop=last)

                for g in range(ng):
                    blk = (g * GT) // TPB
                    if g % CHUNK == 0:
                        stage = stpool.tile([32, CHUNK * P], bf16, tag="stage")
                        nc.gpsimd.dma_start(
                            stage[:],
                            pixT_d[:, g * P:(g + CHUNK) * P])
                    gg2 = g % 2
                    if gg2 == 0:
                        ps = psd.tile([P, 2 * GT * P], f32, space="PSUM")
                    half = slice(gg2 * 512, (gg2 + 1) * 512)
                    nc.tensor.matmul(
                        out=ps[:, half],
                        lhsT=stage[:, (g % CHUNK) * P:(g % CHUNK + 1) * P],
                        rhs=rhsAll[:, 512 * blk:512 * (blk + 1)],
                        start=True, stop=True,
                    )
                    if gg2 == 1:
                        bi = g // 2  # batch index
                        q2 = bi % 2  # parity within the rec pair
                        if q2 == 0:
                            m2n = m2p.tile([P, 16], f32, tag="m2n")
                            pair_scat = []
                        nc.vector.tensor_reduce(
                            out=m2n[:, 8 * q2:8 * q2 + 8],
                            in_=ps[:].rearrange("p (n x) -> p n x", x=P),
                            axis=X, op=Alu.min,
                        )
                        soh = sohp.tile([P, 8 * P], f16)
                        # slots 0..NDVE-1: one strided TT is_equal -> {0,1}
                        nc.vector.tensor_tensor(
                            out=soh[:, 0:NDVE * P].rearrange(
                                "p (n x) -> p n x", x=P),
                            in0=ps[:, 0:NDVE * P].rearrange(
                                "p (n x) -> p n x", x=P),
                            in1=m2n[:, 8 * q2:8 * q2 + NDVE].unsqueeze(
                                2).to_broadcast([P, NDVE, P]),
                            op=Alu.is_equal,
                        )
                        for col in range(8):
                            t = bi * 8 + col
                            mcol = 8 * q2 + col
                            sl = slice(col * P, (col + 1) * P)
                            if col < NDVE:
                                region = acc[:, 128:256]
                                last = (t == t_last_dve)
                            else:
                                # ACT: Sign(m2 - d2) in {0,-1} (0 at the min)
                                nc.scalar.activation(
                                    out=soh[:, sl], in_=ps[:, sl],
                                    func=Act.Sign,
                                    bias=m2n[:, mcol:mcol + 1], scale=-1.0,
                                )
                                region = acc[:, 0:128]
                                last = (t == nt - 1)
                            pair_scat.append((soh[:, sl], 16 * q2 + 2 * col,
                                              region, last))
                        if q2 == 1:
                            pendR.append((bi // 2, m2n, pair_scat))
                            if len(pendR) > 1:
                                pendF.append(rec_chain(pendR.pop(0)))
                            if len(pendF) > 1:
                                flush(pendF.pop(0))
                for entry in pendR:
                    pendF.append(rec_chain(entry))
                for entry in pendF:
                    flush(entry)
                pendR = []
                pendF = []

                # ---- iteration end: AllReduce raw strip, then fixup ----
                accS = smal.tile([2, ACCW], f32, tag="accS")
                nc.scalar.copy(out=accS[:], in_=acc[:, 0:ACCW])
                arin = dram.tile([2, ACCW], f32)
                arout = dram.tile([2, ACCW], f32)
                nc.gpsimd.dma_start(arin[:], accS[:])
                nc.gpsimd.collective_compute(
                    "AllReduce",
                    Alu.add,
                    replica_groups=[list(range(ncores))],
                    ins=[arin[:].opt()],
                    outs=[arout[:].opt()],
                )
                red = smal.tile([2, ACCW], f32, tag="red")
                nc.gpsimd.dma_start(red[:], arout[:])
                # T[d] at partition d: reduce accT pairs, then diag
                t22 = smal.tile([2, 2], f32, tag="t22")
                nc.vector.tensor_reduce(
                    out=t22[:],
                    in_=red[:, 256:256 + 2 * nact].rearrange(
                        "p (t d) -> p d t", d=2),
                    axis=X, op=Alu.add,
                )
                # tsc[p] = t22[p, p] via identity mask + row-sum (no DMA)
                nc.vector.tensor_tensor(out=t22[:], in0=t22[:],
                                        in1=ident[0:2, 0:2], op=Alu.mult)
                tsc = smal.tile([2, 1], f32, tag="tsc")
                nc.vector.tensor_reduce(out=tsc[:], in_=t22[:], axis=X,
                                        op=Alu.add)
                part = smal.tile([2, C], f32, tag="part")
                nc.vector.tensor_scalar(
                    out=part[:], in0=red[:, 0:128], scalar1=1.0,
                    scalar2=tsc[:, 0:1], op0=Alu.mult, op1=Alu.add,
                )
                nc.vector.tensor_tensor(out=part[:], in0=part[:],
                                        in1=red[:, 128:256], op=Alu.add)
                # snap accumulation residues of empty clusters to exact 0 so
                # they collapse to (0,0) duplicates like the reference
                partq = smal.tile([2, C], f32, tag="partq")
                nc.vector.tensor_tensor(out=partq[:], in0=part[:], in1=part[:],
                                        op=Alu.mult)
                nc.vector.tensor_scalar(out=partq[:], in0=partq[:],
                                        scalar1=0.25, scalar2=None,
                                        op0=Alu.is_ge)
                nc.vector.tensor_tensor(out=part[:], in0=part[:], in1=partq[:],
                                        op=Alu.mult)
                ncsp = pse.tile([C, 2], f32, space="PSUM", tag="ep")
                nc.tensor.transpose(out=ncsp[:], in_=part[:],
                                    identity=ident[0:2, 0:2])
                ncs_new = smal.tile([C, 2], f32, tag="ncs_new")
                nc.scalar.copy(out=ncs_new[:], in_=ncsp[:])

                if it == n_iter - 1:
                    nc.gpsimd.dma_start(out_d[:], ncs_new[:])
                else:
                    build_ext(ncs_new)

    nc.compile()
    return nc


def make_core_inputs(core: int, clusters: np.ndarray, heatmap: np.ndarray,
                     nt: int = NT):
    """Host-precomputed per-core constant tables."""
    import ml_dtypes
    bf16 = ml_dtypes.bfloat16

    ng = nt // GT
    nblk = max(1, nt // TPB)
    r0 = core * RPC
    ch = np.float32(r0 + 64)
    hp = (np.arange(P, dtype=np.float32) - np.float32(64.0))
    hw_h2 = hp * hp

    pixT = np.zeros((32, ng * P), np.float32)
    ts = np.arange(nt, dtype=np.float32)
    w0s = (ts // TPB) * TPB + 64.0
    wps = (ts - w0s).astype(np.float32)
    for s in range(GT):
        wp_s = wps[s::GT]                      # [ng]
        cols = pixT[:, :].reshape(32, ng, P)
        cols[8 * s + 0, :, :] = hp[None, :]
        cols[8 * s + 1, :, :] = hp[None, :]
        cols[8 * s + 2, :, :] = wp_s[:, None]
        cols[8 * s + 3, :, :] = wp_s[:, None]
        hw2 = hw_h2[None, :] + (wp_s * wp_s)[:, None]   # [ng, P]
        hi = hw2.astype(bf16).astype(np.float32)
        lo = hw2 - hi
        cols[8 * s + 4, :, :] = hi
        cols[8 * s + 5, :, :] = lo
        cols[8 * s + 6, :, :] = 1.0
        cols[8 * s + 7, :, :] = 1.0
    pixT = pixT.astype(bf16)

    hm = heatmap[r0:r0 + RPC, :nt].astype(np.float32)
    hs = (np.arange(P, dtype=np.float32) + np.float32(r0))
    vhwT = np.empty((P, 2 * nt), np.float32)
    vhwT[:, 0::2] = hs[:, None] * hm
    vhwT[:, 1::2] = np.arange(nt, dtype=np.float32)[None, :] * hm
    vhwT = vhwT.astype(np.float16)

    wB = np.broadcast_to(
        (np.arange(nblk, dtype=np.float32) * TPB + 64.0)[None, :],
        (C, nblk)).copy()

    return {
        "pixT": pixT,
        "vhwT": vhwT,
        "clus": clusters.astype(np.float32),
        "chv": np.full((C, 1), ch, np.float32),
        "wB": wB,
        "ident": np.eye(P, dtype=np.float32),
        "ltri": np.tril(np.ones((P, P), np.float32), -1),
    }


_NC_CACHE = {}


def kernel(clusters: np.ndarray, heatmap: np.ndarray) -> np.ndarray:
    _ensure_repo()
    from concourse.bass_utils import run_bass_kernel_spmd

    clusters = np.asarray(clusters, np.float32)
    heatmap = np.asarray(heatmap, np.float32)

    key = (N_ITER, NT)
    if key not in _NC_CACHE:
        _NC_CACHE[key] = build_nc()
    nc = _NC_CACHE[key]

    in_maps = [make_core_inputs(k, clusters, heatmap) for k in range(NCORES)]
    res = run_bass_kernel_spmd(nc, in_maps, list(range(NCORES)))
    return np.asarray(res.results[0]["out"], np.float32)


if __name__ == "__main__":
    _ensure_repo()
    nc = build_nc(n_iter=int(sys.argv[1]) if len(sys.argv) > 1 else 1,
                  nt=int(sys.argv[2]) if len(sys.argv) > 2 else 128)
    print("built + compiled OK")
